# revision 17
# baseline (speedup 1.0000x reference)
"""Fused CIN-layer kernel for Trainium2 (8 NeuronCores, batch data-parallel).

True reference semantics (derived from the row-major .view + strided conv):
  out[b, n, c*32+t] = sum_{i<32, y<32} W[n,i,y] * x0[b,t,2i+c] * xk[b,y,2i+c] + bias[n]
where c in {0,1} is the f-parity and i indexes f-pairs.

v4.  Per core (128 batches b_l; groups g = (J', c) of 4 b_l's, J' = b_l//4):

  warmup (PE):  5 dummy 512-col matmuls on a memset tile keep the PE
                busy from ~6.5us so the HAM clock-gate lifts the
                1.2GHz cold throttle (-> 2.4GHz) early in stage 1
                instead of ~21us in.
  stage1 (PE):  per quad q (i = 4q+r): 8 matmuls (r x c), each
                G[(c,n), b_l] = sum_y W[n,i,y] * xk[b_l,y,2i+c] via
                tile_position=(32r, 64c); r0/r1 -> gqa (one bank per
                strip), r2/r3 -> gqb.  Evac 2 full-lane copies per quad
                (ACT gqa / DVE gqb) -> gsb[(c,n), b_l*32+i] fp16.
  transpose(PE):per J': [128,128] PE transpose of gsb[:, J'*128:+128]
                -> Gt[(j,i), (c,n)] fp16 PSUM; 4 per gt8 tile, DVE evac
                -> gt_sb[(j,i), g*64+n].
  stage2 (PE):  per g: ONE 128x128 block-diagonal matmul
                out[(j,t), n] = X0bd_g^T @ Gt_g (big matmuls beat
                diagonal row-tiling: 64 LDW+MM pairs, not 256 - the
                per-instruction overhead dominates small matmuls).
  DMA:          win (wst|iden|xks) in 3 chunks on the Sync HWDGE queue
                (FIFO = stage-1 priority); x0a (2MB block-diag) in 3
                chunks on the Scalar HWDGE queue, which starts after
                the ACT-table load, i.e. naturally after win chunk A.
                Output fp16, chunked; last chunk issued from Scalar so
                it doesn't queue behind chunk 6 on Sync.
  PSUM budget:  warm pool closes, stage-1 pool (2 tags x 2 bufs x 2
                banks = 8 banks) closes, then gt/po pools open.
"""

import numpy as np

BS, T, Y, F, NF = 1024, 32, 32, 64, 64
NCORES = 8
BPC = BS // NCORES      # 128 batches per core
NI = 32                 # f-pair index
NQ = NI // 4            # 8 stage-1 quads
NG = 64                 # stage-2 groups g = (J', c), 4 b_l's each
NJP = 32                # J' index (b_l // 4)

W_OFF = 0               # wst at win[:, 0:512]
I_OFF = NQ * NF         # iden at win[:, 512:640]
K_OFF = I_OFF + 128     # xks at win[:, 640:2688]
WIN_W = K_OFF + NQ * 256
# win chunks: [wst|iden|xks q0-1] [xks q2-4] [xks q5-7]
CH1 = K_OFF + 2 * 256
CH2 = K_OFF + 5 * 256

_cached = {}


def _build_bass():
    import concourse.bass as bass
    import concourse.mybir as mybir
    from concourse import bacc
    from concourse.tile import TileContext

    F16 = mybir.dt.float16
    F32 = mybir.dt.float32

    nc = bacc.Bacc()
    win = nc.dram_tensor("win", [128, WIN_W], F16, kind="ExternalInput")
    # block-diagonal x0: partition (j, i); col (J', c, j2, t)
    x0a = nc.dram_tensor("x0a", [128, NG * 128], F16, kind="ExternalInput")
    # out fp16: partition (j, t); col (J', c, n)
    outd = nc.dram_tensor("outd", [128, NG * NF], F16, kind="ExternalOutput")

    with TileContext(nc) as tc:
        with (
            tc.tile_pool(name="const", bufs=1) as cpool,
            tc.tile_pool(name="sb", bufs=1) as spool,
        ):
            win_sb = cpool.tile([128, WIN_W], F16)
            for c0, c1 in ((0, CH1), (CH1, CH2), (CH2, WIN_W)):
                nc.sync.dma_start(out=win_sb[:, c0:c1], in_=win[:, c0:c1])
            x0a_sb = cpool.tile([128, NG * 128], F16)
            # Gate x0a behind win chunk B (the tiny DVE copy reads winB's
            # tail and writes into x0a chunk 1's dst range -> WAW dep):
            # the SDMA engines round-robin all queued transfers, so an
            # ungated x0a starves the stage-1-critical win stream.
            nc.vector.tensor_copy(x0a_sb[0:1, 0:1], win_sb[0:1, CH2 - 1:CH2])
            for c0, c1 in ((0, 2688), (2688, 5440), (5440, 8192)):
                nc.sync.dma_start(out=x0a_sb[:, c0:c1], in_=x0a[:, c0:c1])

            wst_sb = win_sb[:, W_OFF:W_OFF + NQ * NF]
            id_sb = win_sb[0:128, I_OFF:I_OFF + 128]

            gsb = spool.tile([128, BPC * NI], F16)   # G[(c,n), b_l*32+i]
            gt_sb = spool.tile([128, NG * NF], F16)  # Gt[(j,i), g*64+n]
            osb = spool.tile([128, NG * NF], F16)    # out[(j,t), g*64+n]

            # PE warmup: spin the array on a zeroed tile so the HAM
            # activity monitor releases the cold clock throttle before
            # the real work arrives.
            wsrc = spool.tile([32, 128], F16)
            nc.gpsimd.memset(wsrc[:, :], 0)
            with tc.tile_pool(name="warm", bufs=1, space="PSUM") as wpool:
                wps = wpool.tile([64, 512], F32)
                for _ in range(22):
                    nc.tensor.matmul(wps[:, 0:128], wsrc[:, 0:64],
                                     wsrc[:, 0:128], start=True, stop=True)

            # stage 1: 8 quads; quad q covers i = 4q+r; per (r, c) one
            # 32-row matmul at tile_position (32r, 64c)
            # one [128, 2048] fp32 tile = 4 full banks per quad (bank r
            # holds strip r at cols r*512+[0:128]) so all 4 concurrent
            # row strips drain to distinct banks, and the whole quad
            # evacuates in ONE copy (fixed per-copy overhead dominates)
            with tc.tile_pool(name="gq", bufs=2, space="PSUM") as gqpool:
                for q in range(NQ):
                    gq = gqpool.tile([128, 2048], F32, tag="gq")
                    for r in range(4):
                        for c in range(2):
                            nc.tensor.matmul(
                                gq[c * 64:(c + 1) * 64,
                                   r * 512:r * 512 + BPC],
                                wst_sb[32 * r:32 * r + 32, q * NF:(q + 1) * NF],
                                win_sb[32 * r:32 * r + 32,
                                       K_OFF + q * 256 + c * BPC:
                                       K_OFF + q * 256 + (c + 1) * BPC],
                                start=True, stop=True,
                                tile_position=(32 * r, 64 * c),
                            )
                    # evac to gsb[(c,n), b_l*32 + 4q + r]
                    dst = gsb[:, :].rearrange(
                        "p (b i) -> p b i", b=BPC, i=NI)[:, :, 4 * q:4 * q + 4]
                    src = gq[:, :].rearrange(
                        "p (r w) -> p w r", r=4, w=512)[:, :BPC, :]
                    if q % 2 == 0:
                        nc.scalar.copy(dst, src)
                    else:
                        nc.vector.tensor_copy(dst, src)

            with (
                tc.tile_pool(name="gt", bufs=3, space="PSUM") as gtpool,
                tc.tile_pool(name="po", bufs=3, space="PSUM") as popool,
            ):
                # per J8 (8 groups g = 4 J'): 4 transposes -> gt evac ->
                # 8 stage-2 matmuls -> po evac -> chunked output DMA
                for J8 in range(8):
                    # [128, 1024] pads the fp16 tile to a full PSUM bank
                    # so two gt8 bufs never share one (PE-write vs
                    # DVE-read same-bank hazards serialize otherwise)
                    gt8f = gtpool.tile([128, 1024], F16, tag="gt8")
                    gt8 = gt8f[:, 0:512]
                    for s in range(4):
                        Jp = J8 * 4 + s
                        nc.tensor.transpose(
                            gt8[:, s * 128:(s + 1) * 128],
                            gsb[:, Jp * 128:(Jp + 1) * 128],
                            id_sb[:, :],
                        )
                    o0 = J8 * 8 * NF
                    if J8 % 2 == 0:
                        nc.vector.tensor_copy(gt_sb[:, o0:o0 + 512], gt8[:, :])
                    else:
                        nc.scalar.copy(gt_sb[:, o0:o0 + 512], gt8[:, :])

                    po = popool.tile([128, 512], F32, tag="po")
                    for s2 in range(8):
                        g = J8 * 8 + s2
                        nc.tensor.matmul(
                            po[:, s2 * NF:(s2 + 1) * NF],
                            x0a_sb[:, g * 128:(g + 1) * 128],
                            gt_sb[:, g * NF:(g + 1) * NF],
                            start=True, stop=True,
                        )
                    if J8 == 7:
                        # split the last chunk so its first half ships
                        # while the second is still evacuating; issue
                        # from the otherwise-idle GpSimd (SWDGE) queue
                        # so it doesn't wait behind Scalar's evac work
                        nc.vector.tensor_copy(osb[:, o0:o0 + 256],
                                              po[:, 0:256])
                        nc.gpsimd.dma_start(out=outd[:, o0:o0 + 256],
                                            in_=osb[:, o0:o0 + 256])
                        nc.vector.tensor_copy(osb[:, o0 + 256:o0 + 512],
                                              po[:, 256:512])
                        nc.gpsimd.dma_start(out=outd[:, o0 + 256:o0 + 512],
                                            in_=osb[:, o0 + 256:o0 + 512])
                    else:
                        if J8 % 2 == 1:
                            nc.vector.tensor_copy(osb[:, o0:o0 + 512],
                                                  po[:, :])
                        else:
                            nc.scalar.copy(osb[:, o0:o0 + 512], po[:, :])
                    # chunked output: [0,1] [2,3] [4,5] [6] on Sync; the
                    # split chunk [7] rides Scalar (issued above)
                    if J8 in (1, 3, 5):
                        d0 = (J8 - 1) * 8 * NF
                        nc.sync.dma_start(out=outd[:, d0:o0 + 512],
                                          in_=osb[:, d0:o0 + 512])
                    elif J8 == 6:
                        nc.sync.dma_start(out=outd[:, o0:o0 + 512],
                                          in_=osb[:, o0:o0 + 512])
    nc.compile()
    return nc


def _host_prep(x_0, x_k, weight):
    f16 = np.float16
    x_0 = np.asarray(x_0, dtype=np.float32)
    x_k = np.asarray(x_k, dtype=np.float32)
    W = np.asarray(weight, dtype=np.float32).reshape(NF, NI, Y)

    # wst[32r+y, q*64+n] = W[n, 4q+r, y]
    Wr = W.reshape(NF, NQ, 4, Y)                      # n, q, r, y
    wstn = Wr.transpose(2, 3, 1, 0).reshape(128, NQ * NF)
    iden = np.eye(128, dtype=np.float32)

    win_l, x0a_l = [], []
    jj = np.arange(4)
    for core in range(NCORES):
        xkc = x_k[core * BPC:(core + 1) * BPC]        # [128, y, f]
        x0c = x_0[core * BPC:(core + 1) * BPC]        # [128, t, f]
        # xks[32r+y, q*256 + c*128 + b_l] = xk[b_l, y, 2(4q+r)+c]
        xkr = xkc.reshape(BPC, Y, NQ, 4, 2)           # b_l, y, q, r, c
        xksn = xkr.transpose(3, 1, 2, 4, 0).reshape(128, NQ * 256)
        win = np.concatenate([wstn, iden, xksn], axis=1)
        win_l.append(np.ascontiguousarray(win).astype(f16))
        # x0 per (c, J', j): [c, J', j, i, t]
        x0r = x0c.reshape(BPC, T, NI, 2)              # b_l, t, i, c
        A = x0r.transpose(3, 0, 2, 1).reshape(2, NJP, 4, NI, T)
        # block-diagonal: x0bd[c, J', j, i, j2, t] = delta(j,j2)*A[c,J',j,i,t]
        x0bd = np.zeros((2, NJP, 4, NI, 4, T), dtype=np.float32)
        x0bd[:, :, jj, :, jj, :] = A.transpose(2, 0, 1, 3, 4)
        # rows (j, i); cols (J', c, j2, t)
        x0an = x0bd.transpose(2, 3, 1, 0, 4, 5).reshape(128, NG * 128)
        x0a_l.append(np.ascontiguousarray(x0an).astype(f16))

    return win_l, x0a_l


def _in_maps(x_0, x_k, weight):
    win_l, x0a_l = _host_prep(x_0, x_k, weight)
    return [{"win": win_l[c], "x0a": x0a_l[c]} for c in range(NCORES)]


def kernel(x_0, x_k, weight, bias):
    from concourse import bass_utils

    if "nc" not in _cached:
        _cached["nc"] = _build_bass()
    nc = _cached["nc"]

    in_maps = _in_maps(x_0, x_k, weight)
    res = bass_utils.run_bass_kernel_spmd(nc, in_maps, core_ids=list(range(NCORES)))

    bias = np.asarray(bias, dtype=np.float32)
    outs = []
    for c in range(NCORES):
        od = res.results[c]["outd"].astype(np.float32)  # [(j,t), (J',c,n)]
        o = od.reshape(4, T, NJP, 2, NF)            # [j, t, J', c, n]
        o = o.transpose(2, 0, 4, 3, 1)              # [J', j, n, c, t]
        o = o.reshape(BPC, NF, 2 * T)               # [b_l, n, c*32+t]
        outs.append(o)
    out = np.concatenate(outs, axis=0)
    out = out + bias[None, :, None]
    return np.ascontiguousarray(out.astype(np.float32))
